# revision 17
# baseline (speedup 1.0000x reference)
"""Trainium2 Bass kernel for nn_AttentionBlock (GroupNorm -> 1x1 qkv -> full
N^2 attention -> 1x1 proj -> residual) on x:(4, 512, 64, 64).

Sharding: 8 cores = (batch, query-half) pairs. Each core gets one batch's
full image (512 x 4096 pixels) with pixels rotated so that its query half is
always pixels [0:2048]; softmax/attention are permutation-invariant in the
key axis, so every core runs the identical SPMD graph with no collectives.

Key layout choice vs the v1 kernel: attention scores are computed
TRANSPOSED (S^T = K^T-stationary @ Q, keys on partitions, queries on the
free axis). exp(S^T) is then already P^T -- exactly the operand layout
P@V needs (contraction over keys on partitions) -- so the 576 TensorE
transposes and all PSUM->SBUF staging copies of v1 disappear. P@V with
V^T stationary yields O^T (channels on partitions) directly for the proj.
Softmax row sums ride an extra all-ones fp8 stationary matmul into a
dedicated PSUM bank (every output partition gets the same sum); 1/l is
folded into the single PSUM->SBUF copy of O^T.

Numerics (identical scaling scheme to v1): fp8e4 DoubleRow matmuls with
fp32 PSUM; xn8 = 0.5*xn, weights 8x (pre-cast to fp8 on host), q8/k8 =
4*c^-0.25 * q/k, V^T stored 4x, P8 = exp(S - 2.5) with no max-subtraction.
V's bias is folded into the proj bias on the host (softmax rows sum to 1).
GroupNorm group reduction/broadcast uses two tiny one-hot matmuls instead
of transposes; rstd via ScalarE Rsqrt with the exp-table swap hidden in
the qkv phase. Residual is added from the bf16 x copy (no separate f32
residual load).
"""

import numpy as np

C = 512
CB = 4            # 128-channel blocks
N = 4096          # pixels per image
NH = 2048         # query pixels per core
EPS = 1e-6
SCALE = float(C) ** -0.25
FD = 512          # psum free width

_CACHE = {}


def build_bass():
    import concourse.bass as bass
    import concourse.mybir as mybir
    import concourse.tile as tile
    from concourse import bacc
    from concourse.bass import ts

    f32 = mybir.dt.float32
    bf16 = mybir.dt.bfloat16
    fp8 = mybir.dt.float8e4
    AF = mybir.ActivationFunctionType
    ALU = mybir.AluOpType
    AX = mybir.AxisListType
    DR = mybir.MatmulPerfMode.DoubleRow

    nc = bacc.Bacc(None)
    xbf_ext = nc.declare_dram_parameter("xbf", [C, N], bf16, isOutput=False)
    gh_ext = nc.declare_dram_parameter("gammah", [C], f32, isOutput=False)
    bh_ext = nc.declare_dram_parameter("betah", [C], f32, isOutput=False)
    wq8_ext = nc.declare_dram_parameter("wq8", [C, 3 * C], fp8, isOutput=False)
    bqk_ext = nc.declare_dram_parameter("bqk", [2 * C], f32, isOutput=False)
    wp8_ext = nc.declare_dram_parameter("wp8", [C, C], fp8, isOutput=False)
    bp2_ext = nc.declare_dram_parameter("bp2", [C], f32, isOutput=False)
    gm_ext = nc.declare_dram_parameter("gmat", [128, 8], f32, isOutput=False)
    gt_ext = nc.declare_dram_parameter("gtmat", [8, 128], f32, isOutput=False)
    out_ext = nc.declare_dram_parameter("out", [C, NH], f32, isOutput=True)

    with tile.TileContext(nc) as tc:
        with (
            tc.tile_pool(name="const", bufs=1) as cpool,
            tc.tile_pool(name="big", bufs=1) as bigpool,
        ):
            # ---- DMA issues: weights/consts on gpsimd queue, x quarters on
            # four engine queues so descriptor issue parallelizes ----
            gm_sb = cpool.tile([128, 8], f32)
            gt_sb = cpool.tile([8, 128], f32)
            gh_sb = cpool.tile([128, CB], f32)
            bh_sb = cpool.tile([128, CB], f32)
            bqk_sb = cpool.tile([128, 8], f32)
            bp_sb = cpool.tile([128, CB], f32)
            wq8_sb = cpool.tile([128, CB, 3 * C], fp8)
            wp8_sb = cpool.tile([128, CB, C], fp8)
            nc.gpsimd.dma_start(out=gm_sb, in_=gm_ext[0:128, 0:8])
            nc.gpsimd.dma_start(out=gt_sb, in_=gt_ext[0:8, 0:128])
            nc.gpsimd.dma_start(out=gh_sb, in_=gh_ext.rearrange("(t p) -> p t", p=128))
            nc.gpsimd.dma_start(out=bh_sb, in_=bh_ext.rearrange("(t p) -> p t", p=128))
            nc.gpsimd.dma_start(out=bqk_sb, in_=bqk_ext.rearrange("(t p) -> p t", p=128))
            nc.gpsimd.dma_start(out=bp_sb, in_=bp2_ext.rearrange("(t p) -> p t", p=128))
            nc.gpsimd.dma_start(out=wq8_sb, in_=wq8_ext.rearrange("(t p) o -> p t o", p=128))
            nc.gpsimd.dma_start(out=wp8_sb, in_=wp8_ext.rearrange("(t p) o -> p t o", p=128))

            xbf = bigpool.tile([128, CB, N], bf16)
            # half-cc chunks on two parallel queues (4KB descriptors).
            # DVE consumes cc0,cc1,cc2h0; ScalarE consumes cc3,cc2h1.
            for cc, h, eng in (
                (0, 0, nc.sync), (3, 0, nc.scalar),
                (0, 1, nc.sync), (3, 1, nc.scalar),
                (2, 0, nc.sync), (1, 0, nc.scalar),
                (2, 1, nc.sync), (1, 1, nc.scalar),
            ):
                eng.dma_start(
                    out=xbf[:, cc, ts(h, NH)],
                    in_=xbf_ext[cc * 128:(cc + 1) * 128, ts(h, NH)],
                )

            ones8 = cpool.tile([128, 2, 128], fp8)
            nc.vector.memset(ones8, 1.0)
            nbias = cpool.tile([128, 1], f32)
            nc.vector.memset(nbias, -2.5)
            onef = cpool.tile([128, 1], f32)
            nc.vector.memset(onef, 1.0)
            # pre-load the sqrt activation table set off the critical path
            warm0 = cpool.tile([128, 1], f32)
            nc.scalar.activation(out=warm0, in_=onef, func=AF.Sqrt, bias=0.0, scale=1.0)

            # ---- persistent activations ----
            k8 = bigpool.tile([128, CB, N], fp8)
            q8 = bigpool.tile([128, CB, NH], fp8)
            vt = bigpool.tile([128, N // 128, FD], fp8)   # 4*V^T (keys on partitions)
            p8t = bigpool.tile([128, N // 128, FD], fp8)  # P^T for the current chunk

            # ============ phase 1: groupnorm stats (raw sums) ============
            with (
                tc.tile_pool(name="stats", bufs=1) as stpool,
                tc.tile_pool(name="statps", bufs=2, space="PSUM") as pstat,
            ):
                # stat2 cols (cc,f): f0 = mean_c, f1 = E[x^2]_c
                stat2 = stpool.tile([128, 8], f32)
                ssc = stpool.tile([128, NH], bf16)   # ScalarE scratch
                acc3 = stpool.tile([128, 2, 2], f32)  # cc3 (field, half)
                acc2 = stpool.tile([128, 2], f32)     # cc2 h1 (field)
                st_st = stpool.tile([128, 3, 8, 6], f32)
                mv_t = stpool.tile([128, 3, 2], f32)
                tmpm = stpool.tile([128, 8], f32)
                # ScalarE: cc3 (both halves) + cc2 h1 via Identity/Square accum
                for h in range(2):
                    nc.scalar.activation(
                        out=ssc, in_=xbf[:, 3, ts(h, NH)],
                        func=AF.Identity, bias=0.0, scale=1.0,
                        accum_out=acc3[:, 0, h:h + 1],
                    )
                    nc.scalar.activation(
                        out=ssc, in_=xbf[:, 3, ts(h, NH)],
                        func=AF.Square, bias=0.0, scale=1.0,
                        accum_out=acc3[:, 1, h:h + 1],
                    )
                nc.scalar.activation(
                    out=ssc, in_=xbf[:, 2, ts(1, NH)],
                    func=AF.Identity, bias=0.0, scale=1.0,
                    accum_out=acc2[:, 0:1],
                )
                nc.scalar.activation(
                    out=ssc, in_=xbf[:, 2, ts(1, NH)],
                    func=AF.Square, bias=0.0, scale=1.0,
                    accum_out=acc2[:, 1:2],
                )
                # DVE: cc0, cc1 full + cc2 h0 via bn_stats (512-px segments)
                for cc in range(3):
                    nseg = 8 if cc < 2 else 4
                    for s in range(nseg):
                        nc.vector.bn_stats(out=st_st[:, cc, s, :], in_=xbf[:, cc, ts(s, 512)])
                    nc.vector.bn_aggr(out=mv_t[:, cc, :], in_=st_st[:, cc, 0:nseg])
                    nc.vector.tensor_mul(tmpm[:, cc:cc + 1], mv_t[:, cc, 0:1], mv_t[:, cc, 0:1])
                    nc.vector.tensor_tensor(
                        tmpm[:, 4 + cc:5 + cc], mv_t[:, cc, 1:2],
                        tmpm[:, cc:cc + 1], ALU.add,
                    )  # E[x^2] over the bn_stats window
                    if cc < 2:
                        nc.vector.tensor_copy(stat2[:, 2 * cc:2 * cc + 1], mv_t[:, cc, 0:1])
                        nc.vector.tensor_copy(stat2[:, 2 * cc + 1:2 * cc + 2], tmpm[:, 4 + cc:5 + cc])
                # cc2 = 0.5*h0(bn_stats) + h1(Sc accum)/4096... (accum/NH then half)
                nc.vector.tensor_scalar(
                    out=tmpm[:, 7:8], in0=acc2[:, 0:1], scalar1=1.0 / NH,
                    scalar2=mv_t[:, 2, 0:1], op0=ALU.mult, op1=ALU.add,
                )
                nc.vector.tensor_scalar_mul(stat2[:, 4:5], tmpm[:, 7:8], 0.5)
                nc.vector.tensor_scalar(
                    out=tmpm[:, 7:8], in0=acc2[:, 1:2], scalar1=1.0 / NH,
                    scalar2=tmpm[:, 6:7], op0=ALU.mult, op1=ALU.add,
                )
                nc.vector.tensor_scalar_mul(stat2[:, 5:6], tmpm[:, 7:8], 0.5)
                # cc3 from Sc accums
                for f in range(2):
                    nc.vector.tensor_reduce(
                        out=tmpm[:, 7:8], in_=acc3[:, f, :], axis=AX.X, op=ALU.add,
                    )
                    nc.vector.tensor_scalar_mul(stat2[:, 6 + f:7 + f], tmpm[:, 7:8], 1.0 / N)

                # group reduce: one-hot matmul -> [8 groups, (cc,f)]
                gsumT = pstat.tile([8, 8], f32)
                nc.tensor.matmul(gsumT, lhsT=gm_sb, rhs=stat2, start=True, stop=True)
                gv = gsumT.rearrange("p (c f) -> p c f", f=2)
                tmp8 = stpool.tile([8, CB], f32)
                var8 = stpool.tile([8, CB], f32)
                msr = stpool.tile([8, 8], f32)  # cols 0-3 mean, 4-7 rstd
                nc.vector.tensor_copy(msr[:, 0:4], gv[:, :, 0])
                nc.vector.tensor_mul(tmp8, msr[:, 0:4], msr[:, 0:4])
                nc.vector.tensor_tensor(var8, gv[:, :, 1], tmp8, ALU.subtract)
                nc.vector.tensor_scalar_add(var8, var8, EPS)
                nc.vector.reciprocal(var8, var8)
                nc.scalar.activation(
                    out=msr[:, 4:8], in_=var8, func=AF.Sqrt, bias=0.0, scale=1.0
                )
                # broadcast back across partitions: [128, (mean0-3, rstd0-3)]
                mb = pstat.tile([128, 8], f32)
                nc.tensor.matmul(mb, lhsT=gt_sb, rhs=msr, start=True, stop=True)
                sc_sb = stpool.tile([128, CB], f32)   # 0.5*gamma*rstd
                bs_sb = stpool.tile([128, CB], f32)   # 0.5*(beta - mean*gamma*rstd)
                tmpc = stpool.tile([128, CB], f32)
                nc.vector.tensor_mul(sc_sb, gh_sb, mb[:, 4:8])
                nc.vector.tensor_mul(tmpc, mb[:, 0:4], sc_sb)
                nc.vector.tensor_tensor(bs_sb, bh_sb, tmpc, ALU.subtract)

                # ============ phase 2: qkv per 1024-px pair ============
                with (
                    tc.tile_pool(name="xn", bufs=1) as xnpool,
                    tc.tile_pool(name="qkps", bufs=4, space="PSUM") as qps,
                ):
                    xn8 = xnpool.tile([128, CB, N], fp8)
                    for p in range(4):
                        sl = ts(p, 1024)
                        for cc in range(2):
                            nc.vector.tensor_scalar(
                                out=xn8[:, cc, sl], in0=xbf[:, cc, sl],
                                scalar1=sc_sb[:, cc:cc + 1], scalar2=bs_sb[:, cc:cc + 1],
                                op0=ALU.mult, op1=ALU.add,
                            )
                        for cc in range(2, 4):
                            nc.scalar.activation(
                                out=xn8[:, cc, sl], in_=xbf[:, cc, sl],
                                func=AF.Identity, bias=bs_sb[:, cc:cc + 1],
                                scale=sc_sb[:, cc:cc + 1],
                            )
                        for ob in range(CB):  # K first (attention needs all of k8)
                            psa = qps.tile([128, FD], f32, tag="qk", name="qk")
                            psb = qps.tile([128, FD], f32, tag="qk", name="qk")
                            for t in range(2):
                                for sps, sseg in ((psa, 2 * p), (psb, 2 * p + 1)):
                                    nc.tensor.matmul(
                                        sps,
                                        lhsT=wq8_sb[:, 2 * t:2 * t + 2, ts(CB + ob, 128)],
                                        rhs=xn8[:, 2 * t:2 * t + 2, ts(sseg, FD)],
                                        start=(t == 0), stop=(t == 1), perf_mode=DR,
                                    )
                            for sps, sseg in ((psa, 2 * p), (psb, 2 * p + 1)):
                                nc.vector.tensor_scalar(
                                    out=k8[:, ob, ts(sseg, FD)], in0=sps,
                                    scalar1=SCALE, scalar2=bqk_sb[:, CB + ob:CB + ob + 1],
                                    op0=ALU.mult, op1=ALU.add,
                                )
                        if p < 2:  # Q for this core's 2048 query pixels
                            for ob in range(CB):
                                psa = qps.tile([128, FD], f32, tag="qk", name="qk")
                                psb = qps.tile([128, FD], f32, tag="qk", name="qk")
                                for t in range(2):
                                    for sps, sseg in ((psa, 2 * p), (psb, 2 * p + 1)):
                                        nc.tensor.matmul(
                                            sps,
                                            lhsT=wq8_sb[:, 2 * t:2 * t + 2, ts(ob, 128)],
                                            rhs=xn8[:, 2 * t:2 * t + 2, ts(sseg, FD)],
                                            start=(t == 0), stop=(t == 1), perf_mode=DR,
                                        )
                                for sps, sseg in ((psa, 2 * p), (psb, 2 * p + 1)):
                                    nc.vector.tensor_scalar(
                                        out=q8[:, ob, ts(sseg, FD)], in0=sps,
                                        scalar1=SCALE, scalar2=bqk_sb[:, ob:ob + 1],
                                        op0=ALU.mult, op1=ALU.add,
                                    )
                        for jj in range(8):  # V^T (keys on partitions)
                            jb = 8 * p + jj
                            ps = qps.tile([128, FD], f32, tag="qk", name="qk")
                            for t in range(2):
                                nc.tensor.matmul(
                                    ps,
                                    lhsT=xn8[:, 2 * t:2 * t + 2, ts(jb, 128)],
                                    rhs=wq8_sb[:, 2 * t:2 * t + 2, 1024:1536],
                                    start=(t == 0), stop=(t == 1), perf_mode=DR,
                                )
                            if p < 2:  # keep ScalarE free near attention start
                                nc.scalar.activation(
                                    out=vt[:, jb, :], in_=ps,
                                    func=AF.Identity, bias=0.0, scale=1.0,
                                )
                            else:
                                nc.vector.tensor_copy(vt[:, jb, :], ps)
                        if p == 1:
                            # force the exp table swap now, off the critical path
                            warm = stpool.tile([8, CB], f32)
                            nc.scalar.activation(
                                out=warm, in_=var8, func=AF.Exp, bias=0.0, scale=1.0
                            )

            # ========== phase 3: S^T attention + interleaved proj ==========
            with (
                tc.tile_pool(name="fin", bufs=2) as fpool,
                tc.tile_pool(name="sps", bufs=2, space="PSUM") as spool,
                tc.tile_pool(name="ops", bufs=4, space="PSUM") as opool,
                tc.tile_pool(name="lps", bufs=1, space="PSUM") as lpool,
                tc.tile_pool(name="pps", bufs=1, space="PSUM") as ppool,
            ):
                def proj_issue(c, ot_c, pool=None, tag="pj"):
                    pool = pool or ppool
                    for ob in range(CB):
                        pps = pool.tile([128, FD], f32, tag=tag, name=tag)
                        for t in range(2):
                            nc.tensor.matmul(
                                pps,
                                lhsT=wp8_sb[:, 2 * t:2 * t + 2, ts(ob, 128)],
                                rhs=ot_c[:, 2 * t:2 * t + 2, :],
                                start=(t == 0), stop=(t == 1), perf_mode=DR,
                            )
                        y = fpool.tile([128, FD], f32, tag="y", name="y")
                        nc.scalar.activation(
                            out=y, in_=pps, func=AF.Identity,
                            bias=bp_sb[:, ob:ob + 1], scale=1.0 / 32.0,
                        )
                        nc.vector.tensor_tensor(y, y, xbf[:, ob, ts(c, FD)], ALU.add)
                        nc.sync.dma_start(
                            out=out_ext[ob * 128:(ob + 1) * 128, ts(c, FD)], in_=y,
                        )

                prev_pv = None
                prev_fin = None
                pend_proj = None
                for c in range(4):
                    ops = [opool.tile([128, FD], f32, tag="o", name="o") for _ in range(CB)]
                    lps = lpool.tile([128, FD], f32, tag="l", name="l")

                    def pv_issue(jp, ops=ops, lps=lps):
                        for cb in range(CB):
                            nc.tensor.matmul(
                                ops[cb],
                                lhsT=vt[:, 2 * jp:2 * jp + 2, ts(cb, 128)],
                                rhs=p8t[:, 2 * jp:2 * jp + 2, :],
                                start=(jp == 0), stop=(jp == 15), perf_mode=DR,
                            )
                        nc.tensor.matmul(
                            lps, lhsT=ones8, rhs=p8t[:, 2 * jp:2 * jp + 2, :],
                            start=(jp == 0), stop=(jp == 15), perf_mode=DR,
                        )

                    def fin(c=c, ops=ops, lps=lps):
                        rc = fpool.tile([128, FD], f32, tag="rc", name="rc")
                        nc.vector.reciprocal(rc, lps)
                        ot_c = fpool.tile([128, CB, FD], fp8, tag="ot", name="ot")
                        for cb in range(CB):
                            nc.vector.tensor_tensor(ot_c[:, cb, :], ops[cb], rc, ALU.mult)
                        return ot_c

                    for jp in range(16):
                        for jj in range(2):
                            jb = 2 * jp + jj
                            sps = spool.tile([128, FD], f32, tag="s", name="s")
                            for t in range(2):
                                nc.tensor.matmul(
                                    sps,
                                    lhsT=k8[:, 2 * t:2 * t + 2, ts(jb, 128)],
                                    rhs=q8[:, 2 * t:2 * t + 2, ts(c, FD)],
                                    start=(t == 0), stop=(t == 1), perf_mode=DR,
                                )
                            nc.scalar.activation(
                                out=p8t[:, jb, :], in_=sps,
                                func=AF.Exp, bias=nbias, scale=1.0 / 16.0,
                            )
                        if jp == 0 and prev_pv is not None:
                            # drain previous chunk's PV early, then hide its
                            # normalization under our next S^T pairs
                            prev_pv(13)
                            prev_pv(14)
                            prev_pv(15)
                            pend_proj = (c - 1, prev_fin())
                        elif jp >= 3:
                            pv_issue(jp - 3)
                        if jp == 4 and pend_proj is not None:
                            proj_issue(*pend_proj)
                            pend_proj = None
                    prev_pv = pv_issue
                    prev_fin = fin
                for jp in (13, 14, 15):
                    prev_pv(jp)
                proj_issue(3, prev_fin(), pool=opool, tag="o")

    return nc


def _get_nc(finalized: bool):
    key = ("nc", finalized)
    if key not in _CACHE:
        nc = build_bass()
        if finalized:
            nc.finalize()
        _CACHE[key] = nc
    return _CACHE[key]


def make_in_maps(x, gamma, beta, w_qkv, b_qkv, w_proj, b_proj):
    import ml_dtypes

    bf = ml_dtypes.bfloat16
    f8 = ml_dtypes.float8_e4m3
    wq = np.asarray(w_qkv, dtype=np.float32)
    wp = np.asarray(w_proj, dtype=np.float32)
    bq = np.asarray(b_qkv, dtype=np.float32)
    wq8 = np.ascontiguousarray(8.0 * wq.T).astype(f8)
    wp8 = np.ascontiguousarray(8.0 * wp.T).astype(f8)
    bqk = np.ascontiguousarray(4.0 * SCALE * bq[:1024])
    bp2 = np.ascontiguousarray(
        np.asarray(b_proj, dtype=np.float32) + wp @ bq[1024:1536]
    )
    gh = np.ascontiguousarray(0.5 * np.asarray(gamma, dtype=np.float32))
    bh = np.ascontiguousarray(0.5 * np.asarray(beta, dtype=np.float32))
    pidx = np.arange(128) // 16
    gmat = np.zeros((128, 8), dtype=np.float32)
    gmat[np.arange(128), pidx] = 1.0 / 16.0
    gtmat = np.zeros((8, 128), dtype=np.float32)
    gtmat[pidx, np.arange(128)] = 1.0

    in_maps = []
    for core in range(8):
        bb, half = core // 2, core % 2
        xp = np.ascontiguousarray(np.asarray(x[bb], dtype=np.float32).reshape(C, N))
        if half:
            xp = np.ascontiguousarray(np.concatenate([xp[:, NH:], xp[:, :NH]], axis=1))
        in_maps.append(
            {
                "xbf": xp.astype(bf),
                "gammah": gh,
                "betah": bh,
                "wq8": wq8,
                "bqk": bqk,
                "wp8": wp8,
                "bp2": bp2,
                "gmat": gmat,
                "gtmat": gtmat,
            }
        )
    return in_maps


def assemble_out(results, x_dtype=np.float32):
    b = 4
    out = np.zeros((b, C, N), dtype=np.float32)
    for core in range(8):
        bb, half = core // 2, core % 2
        out[bb, :, half * NH:(half + 1) * NH] = results[core]["out"]
    return out.reshape(b, C, 64, 64).astype(x_dtype)


def kernel(x, gamma, beta, w_qkv, b_qkv, w_proj, b_proj):
    from concourse.bass_utils import run_bass_kernel_spmd

    nc = _get_nc(finalized=True)
    in_maps = make_in_maps(x, gamma, beta, w_qkv, b_qkv, w_proj, b_proj)
    res = run_bass_kernel_spmd(nc, in_maps, core_ids=list(range(8)))
    return assemble_out(res.results, np.asarray(x).dtype)


# revision 21
# speedup vs baseline: 1.0883x; 1.0883x over previous
"""Trainium2 Bass kernel for nn_AttentionBlock (GroupNorm -> 1x1 qkv -> full
N^2 attention -> 1x1 proj -> residual) on x:(4, 512, 64, 64).

Sharding: 8 cores = (batch, query-half) pairs. Each core gets one batch's
full image (512 x 4096 pixels) with pixels rotated so that its query half is
always pixels [0:2048]; softmax/attention are permutation-invariant in the
key axis, so every core runs the identical SPMD graph with no collectives.

Key layout choice vs the v1 kernel: attention scores are computed
TRANSPOSED (S^T = K^T-stationary @ Q, keys on partitions, queries on the
free axis). exp(S^T) is then already P^T -- exactly the operand layout
P@V needs (contraction over keys on partitions) -- so the 576 TensorE
transposes and all PSUM->SBUF staging copies of v1 disappear. P@V with
V^T stationary yields O^T (channels on partitions) directly for the proj.
Softmax row sums ride an extra all-ones fp8 stationary matmul into a
dedicated PSUM bank (every output partition gets the same sum); 1/l is
folded into the single PSUM->SBUF copy of O^T.

Numerics (identical scaling scheme to v1): fp8e4 DoubleRow matmuls with
fp32 PSUM; xn8 = 0.5*xn, weights 8x (pre-cast to fp8 on host), q8/k8 =
4*c^-0.25 * q/k, V^T stored 4x, P8 = exp(S - 2.5) with no max-subtraction.
V's bias is folded into the proj bias on the host (softmax rows sum to 1).
GroupNorm group reduction/broadcast uses two tiny one-hot matmuls instead
of transposes; rstd via ScalarE Rsqrt with the exp-table swap hidden in
the qkv phase. Residual is added from the bf16 x copy (no separate f32
residual load).
"""

import numpy as np

C = 512
CB = 4            # 128-channel blocks
N = 4096          # pixels per image
NH = 2048         # query pixels per core
EPS = 1e-6
SCALE = float(C) ** -0.25
FD = 512          # psum free width

_CACHE = {}


def build_bass():
    import concourse.bass as bass
    import concourse.mybir as mybir
    import concourse.tile as tile
    from concourse import bacc
    from concourse.bass import ts

    f32 = mybir.dt.float32
    bf16 = mybir.dt.bfloat16
    fp8 = mybir.dt.float8e4
    AF = mybir.ActivationFunctionType
    ALU = mybir.AluOpType
    AX = mybir.AxisListType
    DR = mybir.MatmulPerfMode.DoubleRow

    nc = bacc.Bacc(None)
    xbf_ext = nc.declare_dram_parameter("xbf", [C, N], bf16, isOutput=False)
    gh_ext = nc.declare_dram_parameter("gammah", [C], f32, isOutput=False)
    bh_ext = nc.declare_dram_parameter("betah", [C], f32, isOutput=False)
    wq8_ext = nc.declare_dram_parameter("wq8", [C, 3 * C], fp8, isOutput=False)
    bqk_ext = nc.declare_dram_parameter("bqk", [2 * C], f32, isOutput=False)
    wp8_ext = nc.declare_dram_parameter("wp8", [C, C], fp8, isOutput=False)
    bp2_ext = nc.declare_dram_parameter("bp2", [C], f32, isOutput=False)
    gm_ext = nc.declare_dram_parameter("gmat", [128, 8], f32, isOutput=False)
    gt_ext = nc.declare_dram_parameter("gtmat", [8, 128], f32, isOutput=False)
    out_ext = nc.declare_dram_parameter("out", [C, NH], f32, isOutput=True)

    with tile.TileContext(nc) as tc:
        with (
            tc.tile_pool(name="const", bufs=1) as cpool,
            tc.tile_pool(name="big", bufs=1) as bigpool,
        ):
            # ---- DMA issues: weights/consts on gpsimd queue, x quarters on
            # four engine queues so descriptor issue parallelizes ----
            gm_sb = cpool.tile([128, 8], f32)
            gt_sb = cpool.tile([8, 128], f32)
            gh_sb = cpool.tile([128, CB], f32)
            bh_sb = cpool.tile([128, CB], f32)
            bqk_sb = cpool.tile([128, 8], f32)
            bp_sb = cpool.tile([128, CB], f32)
            wq8_sb = cpool.tile([128, CB, 3 * C], fp8)
            wp8_sb = cpool.tile([128, CB, C], fp8)
            nc.gpsimd.dma_start(out=gm_sb, in_=gm_ext[0:128, 0:8])
            nc.gpsimd.dma_start(out=gt_sb, in_=gt_ext[0:8, 0:128])
            nc.gpsimd.dma_start(out=gh_sb, in_=gh_ext.rearrange("(t p) -> p t", p=128))
            nc.gpsimd.dma_start(out=bh_sb, in_=bh_ext.rearrange("(t p) -> p t", p=128))
            nc.gpsimd.dma_start(out=bqk_sb, in_=bqk_ext.rearrange("(t p) -> p t", p=128))
            nc.gpsimd.dma_start(out=bp_sb, in_=bp2_ext.rearrange("(t p) -> p t", p=128))
            nc.gpsimd.dma_start(out=wq8_sb, in_=wq8_ext.rearrange("(t p) o -> p t o", p=128))
            nc.gpsimd.dma_start(out=wp8_sb, in_=wp8_ext.rearrange("(t p) o -> p t o", p=128))

            xbf = bigpool.tile([128, CB, N], bf16)
            # full-cc chunks (8KB descriptors sustain ~2x the DMA rate of 4KB).
            # DVE consumes cc0,cc1,cc2h0; ScalarE consumes cc3,cc2h1.
            for cc, eng in ((0, nc.sync), (3, nc.scalar), (2, nc.sync), (1, nc.scalar)):
                eng.dma_start(
                    out=xbf[:, cc, :],
                    in_=xbf_ext[cc * 128:(cc + 1) * 128, :],
                )

            ones8 = cpool.tile([128, 2, 128], fp8)
            nc.vector.memset(ones8, 1.0)
            nbias = cpool.tile([128, 1], f32)
            nc.vector.memset(nbias, -2.5)
            onef = cpool.tile([128, 1], f32)
            nc.vector.memset(onef, 1.0)
            # pre-load the sqrt activation table set off the critical path
            warm0 = cpool.tile([128, 1], f32)
            nc.scalar.activation(out=warm0, in_=onef, func=AF.Sqrt, bias=0.0, scale=1.0)

            # ---- persistent activations ----
            k8 = bigpool.tile([128, CB, N], fp8)
            q8 = bigpool.tile([128, CB, NH], fp8)
            vt = bigpool.tile([128, N // 128, FD], fp8)   # 4*V^T (keys on partitions)
            p8t = bigpool.tile([128, N // 128, FD], fp8)  # P^T for the current chunk

            # ============ phase 1: groupnorm stats (raw sums) ============
            with (
                tc.tile_pool(name="stats", bufs=1) as stpool,
                tc.tile_pool(name="statps", bufs=1, space="PSUM") as pstat,
            ):
                # stat2 cols (cc,f): f0 = mean_c, f1 = E[x^2]_c
                stat2 = stpool.tile([128, 8], f32)
                ssc = stpool.tile([128, NH], bf16)   # ScalarE scratch
                acc3 = stpool.tile([128, 2, 2], f32)  # cc3 (field, half)
                acc2 = stpool.tile([128, 2], f32)     # cc2 h1 (field)
                st_st = stpool.tile([128, 3, 8, 6], f32)
                mv_t = stpool.tile([128, 3, 2], f32)
                tmpm = stpool.tile([128, 8], f32)
                # ScalarE: cc3 (both halves) + cc2 h1 via Identity/Square accum
                for h in range(2):
                    nc.scalar.activation(
                        out=ssc, in_=xbf[:, 3, ts(h, NH)],
                        func=AF.Identity, bias=0.0, scale=1.0,
                        accum_out=acc3[:, 0, h:h + 1],
                    )
                    nc.scalar.activation(
                        out=ssc, in_=xbf[:, 3, ts(h, NH)],
                        func=AF.Square, bias=0.0, scale=1.0,
                        accum_out=acc3[:, 1, h:h + 1],
                    )
                nc.scalar.activation(
                    out=ssc, in_=xbf[:, 2, ts(1, NH)],
                    func=AF.Identity, bias=0.0, scale=1.0,
                    accum_out=acc2[:, 0:1],
                )
                nc.scalar.activation(
                    out=ssc, in_=xbf[:, 2, ts(1, NH)],
                    func=AF.Square, bias=0.0, scale=1.0,
                    accum_out=acc2[:, 1:2],
                )
                # DVE: cc0, cc1 full + cc2 h0 via bn_stats (512-px segments)
                for cc in range(3):
                    nseg = 8 if cc < 2 else 4
                    for s in range(nseg):
                        nc.vector.bn_stats(out=st_st[:, cc, s, :], in_=xbf[:, cc, ts(s, 512)])
                    nc.vector.bn_aggr(out=mv_t[:, cc, :], in_=st_st[:, cc, 0:nseg])
                    nc.vector.tensor_mul(tmpm[:, cc:cc + 1], mv_t[:, cc, 0:1], mv_t[:, cc, 0:1])
                    nc.vector.tensor_tensor(
                        tmpm[:, 4 + cc:5 + cc], mv_t[:, cc, 1:2],
                        tmpm[:, cc:cc + 1], ALU.add,
                    )  # E[x^2] over the bn_stats window
                    if cc < 2:
                        nc.vector.tensor_copy(stat2[:, 2 * cc:2 * cc + 1], mv_t[:, cc, 0:1])
                        nc.vector.tensor_copy(stat2[:, 2 * cc + 1:2 * cc + 2], tmpm[:, 4 + cc:5 + cc])
                # cc2 = 0.5*h0(bn_stats) + h1(Sc accum)/4096... (accum/NH then half)
                nc.vector.tensor_scalar(
                    out=tmpm[:, 7:8], in0=acc2[:, 0:1], scalar1=1.0 / NH,
                    scalar2=mv_t[:, 2, 0:1], op0=ALU.mult, op1=ALU.add,
                )
                nc.vector.tensor_scalar_mul(stat2[:, 4:5], tmpm[:, 7:8], 0.5)
                nc.vector.tensor_scalar(
                    out=tmpm[:, 7:8], in0=acc2[:, 1:2], scalar1=1.0 / NH,
                    scalar2=tmpm[:, 6:7], op0=ALU.mult, op1=ALU.add,
                )
                nc.vector.tensor_scalar_mul(stat2[:, 5:6], tmpm[:, 7:8], 0.5)
                # cc3 from Sc accums
                for f in range(2):
                    nc.vector.tensor_reduce(
                        out=tmpm[:, 7:8], in_=acc3[:, f, :], axis=AX.X, op=ALU.add,
                    )
                    nc.vector.tensor_scalar_mul(stat2[:, 6 + f:7 + f], tmpm[:, 7:8], 1.0 / N)

                # group reduce: one-hot matmul -> [8 groups, (cc,f)]
                gsumT = pstat.tile([8, 8], f32)
                nc.tensor.matmul(gsumT, lhsT=gm_sb, rhs=stat2, start=True, stop=True)
                gv = gsumT.rearrange("p (c f) -> p c f", f=2)
                tmp8 = stpool.tile([8, CB], f32)
                var8 = stpool.tile([8, CB], f32)
                msr = stpool.tile([8, 8], f32)  # cols 0-3 mean, 4-7 rstd
                nc.vector.tensor_copy(msr[:, 0:4], gv[:, :, 0])
                nc.vector.tensor_mul(tmp8, msr[:, 0:4], msr[:, 0:4])
                nc.vector.tensor_tensor(var8, gv[:, :, 1], tmp8, ALU.subtract)
                nc.vector.tensor_scalar_add(var8, var8, EPS)
                nc.vector.reciprocal(var8, var8)
                nc.scalar.activation(
                    out=msr[:, 4:8], in_=var8, func=AF.Sqrt, bias=0.0, scale=1.0
                )
                # broadcast back across partitions: [128, (mean0-3, rstd0-3)]
                mb = pstat.tile([128, 8], f32)
                nc.tensor.matmul(mb, lhsT=gt_sb, rhs=msr, start=True, stop=True)
                sc_sb = stpool.tile([128, CB], f32)   # 0.5*gamma*rstd
                bs_sb = stpool.tile([128, CB], f32)   # 0.5*(beta - mean*gamma*rstd)
                tmpc = stpool.tile([128, CB], f32)
                nc.vector.tensor_mul(sc_sb, gh_sb, mb[:, 4:8])
                nc.vector.tensor_mul(tmpc, mb[:, 0:4], sc_sb)
                nc.vector.tensor_tensor(bs_sb, bh_sb, tmpc, ALU.subtract)

                # ============ phase 2: qkv per 1024-px pair ============
                with (
                    tc.tile_pool(name="xn", bufs=1) as xnpool,
                    tc.tile_pool(name="qkps", bufs=2, space="PSUM") as qps,
                ):
                    xn8 = xnpool.tile([128, CB, N], fp8)
                    for p in range(4):
                        sl = ts(p, 1024)
                        for cc in range(2):
                            nc.vector.tensor_scalar(
                                out=xn8[:, cc, sl], in0=xbf[:, cc, sl],
                                scalar1=sc_sb[:, cc:cc + 1], scalar2=bs_sb[:, cc:cc + 1],
                                op0=ALU.mult, op1=ALU.add,
                            )
                        for cc in range(2, 4):
                            nc.scalar.activation(
                                out=xn8[:, cc, sl], in_=xbf[:, cc, sl],
                                func=AF.Identity, bias=bs_sb[:, cc:cc + 1],
                                scale=sc_sb[:, cc:cc + 1],
                            )
                        for ob in range(CB):  # K first (attention needs all of k8)
                            pq = qps.tile([128, 2, FD], f32, tag="qk", name="qk")
                            for t in range(2):
                                for ss in range(2):
                                    nc.tensor.matmul(
                                        pq[:, ss, :],
                                        lhsT=wq8_sb[:, 2 * t:2 * t + 2, ts(CB + ob, 128)],
                                        rhs=xn8[:, 2 * t:2 * t + 2, ts(2 * p + ss, FD)],
                                        start=(t == 0), stop=(t == 1), perf_mode=DR,
                                    )
                            nc.scalar.activation(
                                out=k8[:, ob, ts(p, 1024)],
                                in_=pq.rearrange("p s f -> p (s f)"),
                                func=AF.Identity, bias=bqk_sb[:, CB + ob:CB + ob + 1],
                                scale=SCALE,
                            )
                        if p < 2:  # Q for this core's 2048 query pixels
                            for ob in range(CB):
                                pq = qps.tile([128, 2, FD], f32, tag="qk", name="qk")
                                for t in range(2):
                                    for ss in range(2):
                                        nc.tensor.matmul(
                                            pq[:, ss, :],
                                            lhsT=wq8_sb[:, 2 * t:2 * t + 2, ts(ob, 128)],
                                            rhs=xn8[:, 2 * t:2 * t + 2, ts(2 * p + ss, FD)],
                                            start=(t == 0), stop=(t == 1), perf_mode=DR,
                                        )
                                nc.vector.tensor_scalar(
                                    out=q8[:, ob, ts(p, 1024)],
                                    in0=pq.rearrange("p s f -> p (s f)"),
                                    scalar1=SCALE, scalar2=bqk_sb[:, ob:ob + 1],
                                    op0=ALU.mult, op1=ALU.add,
                                )
                        for jj in range(8):  # V^T (keys on partitions)
                            jb = 8 * p + jj
                            ps = qps.tile([128, FD], f32, tag="v", name="v")
                            for t in range(2):
                                nc.tensor.matmul(
                                    ps,
                                    lhsT=xn8[:, 2 * t:2 * t + 2, ts(jb, 128)],
                                    rhs=wq8_sb[:, 2 * t:2 * t + 2, 1024:1536],
                                    start=(t == 0), stop=(t == 1), perf_mode=DR,
                                )
                            nc.vector.tensor_copy(vt[:, jb, :], ps)
                        if p == 1:
                            # force the exp table swap now, off the critical path
                            warm = stpool.tile([8, CB], f32)
                            nc.scalar.activation(
                                out=warm, in_=var8, func=AF.Exp, bias=0.0, scale=1.0
                            )

            # ========== phase 3: S^T attention + interleaved proj ==========
            with (
                tc.tile_pool(name="fin", bufs=2) as fpool,
                tc.tile_pool(name="sps", bufs=2, space="PSUM") as spool,
                tc.tile_pool(name="ops", bufs=4, space="PSUM") as opool,
                tc.tile_pool(name="lps", bufs=1, space="PSUM") as lpool,
                tc.tile_pool(name="pps", bufs=1, space="PSUM") as ppool,
            ):
                def proj_issue(c, ot_c, pool=None, tag="pj"):
                    pool = pool or ppool
                    for ob in range(CB):
                        pps = pool.tile([128, FD], f32, tag=tag, name=tag)
                        for t in range(2):
                            nc.tensor.matmul(
                                pps,
                                lhsT=wp8_sb[:, 2 * t:2 * t + 2, ts(ob, 128)],
                                rhs=ot_c[:, 2 * t:2 * t + 2, :],
                                start=(t == 0), stop=(t == 1), perf_mode=DR,
                            )
                        y = fpool.tile([128, FD], f32, tag="y", name="y")
                        nc.scalar.activation(
                            out=y, in_=pps, func=AF.Identity,
                            bias=bp_sb[:, ob:ob + 1], scale=1.0 / 32.0,
                        )
                        nc.vector.tensor_tensor(y, y, xbf[:, ob, ts(c, FD)], ALU.add)
                        nc.sync.dma_start(
                            out=out_ext[ob * 128:(ob + 1) * 128, ts(c, FD)], in_=y,
                        )

                prev_pv = None
                prev_fin = None
                pend_proj = None
                for c in range(4):
                    ops = [opool.tile([128, FD], f32, tag="o", name="o") for _ in range(CB)]
                    lps = lpool.tile([128, FD], f32, tag="l", name="l")

                    def pv_issue(jp, ops=ops, lps=lps):
                        for cb in range(CB):
                            nc.tensor.matmul(
                                ops[cb],
                                lhsT=vt[:, 2 * jp:2 * jp + 2, ts(cb, 128)],
                                rhs=p8t[:, 2 * jp:2 * jp + 2, :],
                                start=(jp == 0), stop=(jp == 15), perf_mode=DR,
                            )
                        nc.tensor.matmul(
                            lps, lhsT=ones8, rhs=p8t[:, 2 * jp:2 * jp + 2, :],
                            start=(jp == 0), stop=(jp == 15), perf_mode=DR,
                        )

                    def fin(c=c, ops=ops, lps=lps):
                        rc = fpool.tile([128, FD], f32, tag="rc", name="rc")
                        nc.vector.reciprocal_approx_fast(out=rc, in_=lps)
                        ot_c = fpool.tile([128, CB, FD], fp8, tag="ot", name="ot")
                        for cb in range(CB):
                            nc.vector.tensor_tensor(ot_c[:, cb, :], ops[cb], rc, ALU.mult)
                        return ot_c

                    for jp in range(16):
                        for jj in range(2):
                            jb = 2 * jp + jj
                            sps = spool.tile([128, FD], f32, tag="s", name="s")
                            for t in range(2):
                                nc.tensor.matmul(
                                    sps,
                                    lhsT=k8[:, 2 * t:2 * t + 2, ts(jb, 128)],
                                    rhs=q8[:, 2 * t:2 * t + 2, ts(c, FD)],
                                    start=(t == 0), stop=(t == 1), perf_mode=DR,
                                )
                            nc.scalar.activation(
                                out=p8t[:, jb, :], in_=sps,
                                func=AF.Exp, bias=nbias, scale=1.0 / 16.0,
                            )
                        if jp == 0 and prev_pv is not None:
                            # drain previous chunk's PV early, then hide its
                            # normalization under our next S^T pairs
                            prev_pv(13)
                            prev_pv(14)
                            prev_pv(15)
                            pend_proj = (c - 1, prev_fin())
                        elif jp >= 3:
                            pv_issue(jp - 3)
                        if jp == 4 and pend_proj is not None:
                            proj_issue(*pend_proj)
                            pend_proj = None
                    prev_pv = pv_issue
                    prev_fin = fin
                for jp in (13, 14, 15):
                    prev_pv(jp)
                proj_issue(3, prev_fin(), pool=opool, tag="o")

    return nc


def _get_nc(finalized: bool):
    key = ("nc", finalized)
    if key not in _CACHE:
        nc = build_bass()
        if finalized:
            nc.finalize()
        _CACHE[key] = nc
    return _CACHE[key]


def make_in_maps(x, gamma, beta, w_qkv, b_qkv, w_proj, b_proj):
    import ml_dtypes

    bf = ml_dtypes.bfloat16
    f8 = ml_dtypes.float8_e4m3
    wq = np.asarray(w_qkv, dtype=np.float32)
    wp = np.asarray(w_proj, dtype=np.float32)
    bq = np.asarray(b_qkv, dtype=np.float32)
    wq8 = np.ascontiguousarray(8.0 * wq.T).astype(f8)
    wp8 = np.ascontiguousarray(8.0 * wp.T).astype(f8)
    bqk = np.ascontiguousarray(4.0 * SCALE * bq[:1024])
    bp2 = np.ascontiguousarray(
        np.asarray(b_proj, dtype=np.float32) + wp @ bq[1024:1536]
    )
    gh = np.ascontiguousarray(0.5 * np.asarray(gamma, dtype=np.float32))
    bh = np.ascontiguousarray(0.5 * np.asarray(beta, dtype=np.float32))
    pidx = np.arange(128) // 16
    gmat = np.zeros((128, 8), dtype=np.float32)
    gmat[np.arange(128), pidx] = 1.0 / 16.0
    gtmat = np.zeros((8, 128), dtype=np.float32)
    gtmat[pidx, np.arange(128)] = 1.0

    in_maps = []
    for core in range(8):
        bb, half = core // 2, core % 2
        xp = np.ascontiguousarray(np.asarray(x[bb], dtype=np.float32).reshape(C, N))
        if half:
            xp = np.ascontiguousarray(np.concatenate([xp[:, NH:], xp[:, :NH]], axis=1))
        in_maps.append(
            {
                "xbf": xp.astype(bf),
                "gammah": gh,
                "betah": bh,
                "wq8": wq8,
                "bqk": bqk,
                "wp8": wp8,
                "bp2": bp2,
                "gmat": gmat,
                "gtmat": gtmat,
            }
        )
    return in_maps


def assemble_out(results, x_dtype=np.float32):
    b = 4
    out = np.zeros((b, C, N), dtype=np.float32)
    for core in range(8):
        bb, half = core // 2, core % 2
        out[bb, :, half * NH:(half + 1) * NH] = results[core]["out"]
    return out.reshape(b, C, 64, 64).astype(x_dtype)


def kernel(x, gamma, beta, w_qkv, b_qkv, w_proj, b_proj):
    from concourse.bass_utils import run_bass_kernel_spmd

    nc = _get_nc(finalized=True)
    in_maps = make_in_maps(x, gamma, beta, w_qkv, b_qkv, w_proj, b_proj)
    res = run_bass_kernel_spmd(nc, in_maps, core_ids=list(range(8)))
    return assemble_out(res.results, np.asarray(x).dtype)
